# revision 18
# baseline (speedup 1.0000x reference)
"""ColorHistogramLoss Trainium2 kernel.

Problem: loss = mean(|hist(input) - hist(target)|) with 64-bin histograms
per (batch, channel) over [-1, 1), inputs [32, 3, 512, 512] f32.

Strategy (8 cores, data-parallel over batch, 4 batches/core):
  - Binning: w = bf16_rne(v*(63/128) + (191/128 - 2^-8)). The -2^-8 pre-bias
    turns bf16 round-to-nearest into floor onto the 2^-7 grid of [1,2), so
    (w >= 1 + j/64) reproduces searchsorted binning exactly (boundary-rounding
    differences ~1e-5 of elements, loss rel-err ~1e-4).
  - CDF counts per edge j, split across three engines (HW-measured balance):
      ACT: activation(Sign, bias=-(1+j/64-2^-9), accum_out) — fused count.
      DVE: tensor_scalar(is_ge, imm) WITHOUT accum (keeps the 4x perf mode;
           the fused accum variant drops to 2x) producing a 0/1 bf16 mask.
      PE:  reduces each mask: matmul(ones-indicator [128,4] x mask chunk
           [128,512]) accumulated over 16 chunks into PSUM [4 imgs, 512];
           8 edges share one [32,512] PSUM tile (disjoint 4-row slices).
      A tiny DVE accum op then reduces each PSUM tile to per-(edge,img)
      counts. Exact integer arithmetic throughout (f32 PSUM, counts < 2^24).
  - Host differentiates the CDF and does the tiny final reduction.
  - Layout: 24 channel-images per core (4 batches x 3 ch x 2 tensors),
    packed 4 per SBUF tile as [128, 8192] f32 -> 6 group tiles.
"""

import numpy as np
import ml_dtypes

BINS = 64
N_CORES = 8
B, C, H, W = 32, 3, 512, 512
NPIX = H * W                  # 262144 per channel-image
B_LOC = B // N_CORES          # 4
IMGS = 2 * B_LOC * C          # 24 channel-images per core
PACK = 4                      # channel-images per SBUF group tile
GROUPS = IMGS // PACK         # 6
PART_PER_IMG = 128 // PACK    # 32 partitions per image
FD = NPIX // PART_PER_IMG     # 8192 free-dim elements per partition

SCALE = float(np.float32(63.0 / 128.0))              # exact in f32
BIAS2 = float(np.float32(191.0 / 128.0) - np.float32(2.0 ** -8))

# edges j=1..63; ACT (Sign+accum) takes the first N_ACT, the rest go
# DVE-mask + PE-reduce. HW A/B (For_i slope timing; axon dispatch jitter
# makes single-shot timing useless): fused-accum DVE runs ~8.3us/tile (2x
# cap + DRAIN) vs ~4.2us for mask-only (4x), ACT ~10us/tile, PE reduction
# ~3.4us/mask — so PE absorbs the accumulation and DVE produces masks.
# Full-workload slopes: all-fused 16/47 split 2.87ms, 28/35 fused 1.86ms,
# PE-reduce n_act=8 1.21ms; + ACT-on-raw-input + conv/readouts on ACT +
# masks in 2 half-tiles (better DVE->PE overlap) 1.05ms.
N_ACT = 8
MASK_SPLIT = 2                # mask half-tiles per edge (DVE->PE overlap)
EDGE_BLOCK = 8                # PE-routed edges per PSUM tile (4 rows each)
N_PE = BINS - 1 - N_ACT       # 44
N_BLOCKS = (N_PE + EDGE_BLOCK - 1) // EDGE_BLOCK     # 6
CHUNK = 512                   # matmul moving free size
N_CHUNKS = FD // CHUNK        # 16

_cache = {}


def _build():
    from concourse import bacc
    import concourse.mybir as mybir
    from concourse.tile import TileContext

    f32 = mybir.dt.float32
    bf16 = mybir.dt.bfloat16

    nc = bacc.Bacc("TRN2", target_bir_lowering=False, debug=False,
                   num_devices=N_CORES)
    x = nc.declare_dram_parameter("x", [GROUPS, 128, FD], f32, isOutput=False)
    bias_a = nc.declare_dram_parameter(
        "bias_a", [128, N_ACT], f32, isOutput=False)
    # 8 stationary variants [128, 32]: variant e holds the 4 per-image
    # indicator columns at columns 4e..4e+4 (zeros elsewhere), so every
    # matmul targets the full [32, CHUNK] PSUM tile (PE requires output
    # base partition 0/32/64) and cross-edge rows just accumulate zeros.
    wones_p = nc.declare_dram_parameter(
        "wones", [128, 32 * EDGE_BLOCK], mybir.dt.bfloat16, isOutput=False)
    # counts_a[g, p, j] = sum(sign(w - (1 + j/64 - 2^-9))) for ACT-owned j
    counts_a = nc.declare_dram_parameter(
        "counts_a", [GROUPS, 128, BINS], f32, isOutput=True)
    # counts_pe[g, 4*e+i, b] = #{w >= edge(block b, slot e)} in image i
    counts_pe = nc.declare_dram_parameter(
        "counts_pe", [GROUPS, 32, N_BLOCKS], f32, isOutput=True)

    edges = list(range(1, BINS))
    edges_act = edges[:N_ACT]
    edges_pe = edges[N_ACT:]

    with TileContext(nc) as tc, tc.tile_pool(name="p", bufs=2) as pool, \
            tc.psum_pool(name="pp", bufs=2) as ppool:
        bt = pool.tile([128, N_ACT], f32, tag="bt")
        nc.sync.dma_start(out=bt[:], in_=bias_a[:])
        wones = pool.tile([128, 32 * EDGE_BLOCK], bf16, tag="wones")
        nc.sync.dma_start(out=wones[:], in_=wones_p[:])
        HFD = FD // MASK_SPLIT
        HCH = HFD // CHUNK
        for g in range(GROUPS):
            vt = pool.tile([128, FD], f32, tag="vt")
            nc.sync.dma_start(out=vt[:], in_=x[g])
            # conv on ACT (DVE is the bottleneck engine): w = bf16(v*S + B)
            w = pool.tile([128, FD], bf16, tag="w")
            nc.scalar.activation(
                out=w[:], in_=vt[:],
                func=mybir.ActivationFunctionType.Copy,
                bias=BIAS2, scale=SCALE)

            cnt_a = pool.tile([128, BINS], f32, tag="cnta")
            nc.gpsimd.memset(cnt_a[:], 0.0)
            mask_a = pool.tile([128, FD], bf16, tag="maska")
            # ACT compares raw v directly: sign(v*S + (B - t_j + 2^-8)) == 1
            # iff RNE(v*S + B) >= t_j (tie at t_j - 2^-8 rounds up to the
            # even mantissa t_j), matching the w-grid compare exactly.
            for k, j in enumerate(edges_act):
                nc.scalar.activation(
                    out=mask_a[:], in_=vt[:],
                    func=mybir.ActivationFunctionType.Sign,
                    bias=bt[:, k:k + 1], scale=SCALE,
                    accum_out=cnt_a[:, j:j + 1])

            cnt_pe = pool.tile([32, N_BLOCKS], f32, tag="cntpe")
            scratch = pool.tile([32, CHUNK], bf16, tag="scratch")
            for b in range(N_BLOCKS):
                blk = edges_pe[b * EDGE_BLOCK:(b + 1) * EDGE_BLOCK]
                ps = ppool.tile([32, CHUNK], f32, tag="ps", name="ps")
                for e, j in enumerate(blk):
                    # One accumulation group per edge-half (the scheduler
                    # treats a group as atomic on PE; a group spanning all 8
                    # edges would wait on masks produced mid-group ->
                    # deadlock). Groups after the first accumulate onto ps
                    # via start=False; cross-edge rows only ever add zeros.
                    # Masks are produced in MASK_SPLIT half-tiles so the PE
                    # starts reducing half h while DVE writes half h+1.
                    for h in range(MASK_SPLIT):
                        mask = pool.tile([128, HFD], bf16, tag=f"mask{h}",
                                         name=f"mask{h}")
                        nc.vector.tensor_scalar(
                            out=mask[:], in0=w[:, h * HFD:(h + 1) * HFD],
                            scalar1=float(1.0 + j / 64.0), scalar2=None,
                            op0=mybir.AluOpType.is_ge,
                            op1=mybir.AluOpType.bypass)
                        for c in range(HCH):
                            nc.tensor.matmul(
                                out=ps[:, :],
                                lhsT=wones[:, 32 * e:32 * (e + 1)],
                                rhs=mask[:, c * CHUNK:(c + 1) * CHUNK],
                                start=(e == 0 and h == 0 and c == 0),
                                stop=(h == MASK_SPLIT - 1 and c == HCH - 1),
                                skip_group_check=True)
                # rows 4e..4e+4 of ps hold per-image chunk-partial sums for
                # edge blk[e]; reduce the 512 columns into one count each
                # (on ACT — keeps the bottleneck DVE mask-only).
                nrows = 4 * len(blk)
                nc.scalar.activation(
                    out=scratch[:nrows, :], in_=ps[:nrows, :],
                    func=mybir.ActivationFunctionType.Copy,
                    bias=0.0, scale=1.0,
                    accum_out=cnt_pe[:nrows, b:b + 1])
            nc.sync.dma_start(out=counts_a[g], in_=cnt_a[:])
            nc.sync.dma_start(out=counts_pe[g], in_=cnt_pe[:])
    nc.finalize()
    return nc


def _get_nc():
    if "nc" not in _cache:
        _cache["nc"] = _build()
    return _cache["nc"]


def _pack_core(inp_c: np.ndarray, tgt_c: np.ndarray) -> np.ndarray:
    """[4,3,512,512] x2 f32 -> [GROUPS, 128, FD]; image i = t*12 + b*3 + c."""
    imgs = np.concatenate(
        [inp_c.reshape(B_LOC * C, NPIX), tgt_c.reshape(B_LOC * C, NPIX)], axis=0)
    return np.ascontiguousarray(
        imgs.reshape(GROUPS, PACK, PART_PER_IMG, FD).reshape(GROUPS, 128, FD))


def _counts_to_loss(results) -> np.float32:
    """results: list of 8 dicts with counts_a [G,128,BINS], counts_pe
    [G,32,N_BLOCKS]."""
    total = np.float64(0.0)
    for c in range(N_CORES):
        ca = np.asarray(results[c]["counts_a"], np.float64)
        cpe = np.asarray(results[c]["counts_pe"], np.float64)
        ca = ca.reshape(GROUPS, PACK, PART_PER_IMG, BINS).sum(axis=2)
        ca = ca.reshape(IMGS, BINS)
        cdf = np.zeros((IMGS, BINS), np.float64)
        cdf[:, 0] = NPIX
        for j in range(1, 1 + N_ACT):
            cdf[:, j] = (NPIX + ca[:, j]) / 2.0   # sign-sum -> count_ge
        for k in range(N_PE):
            j = 1 + N_ACT + k
            b, e = divmod(k, EDGE_BLOCK)
            for g in range(GROUPS):
                for i in range(PACK):
                    cdf[g * PACK + i, j] = cpe[g, 4 * e + i, b]
        counts = np.empty((IMGS, BINS), np.float64)
        counts[:, :-1] = cdf[:, :-1] - cdf[:, 1:]
        counts[:, -1] = cdf[:, -1]
        hist = counts / NPIX   # [24, 64]; images 0..11 = input, 12..23 = target
        h_in = hist[: B_LOC * C].reshape(B_LOC, C * BINS)
        h_tg = hist[B_LOC * C:].reshape(B_LOC, C * BINS)
        total += np.abs(h_in - h_tg).sum()
    return np.float32(total / (B * C * BINS))


def _bias_np() -> np.ndarray:
    # ACT reads raw v: sign(v*SCALE + bias_j) >= 0  <=>  x >= t_j - 2^-8
    # with x = v*SCALE + BIAS2, i.e. RNE_bf16(x) >= t_j (tie rounds up).
    cols = [float(np.float32(BIAS2) - np.float32(1.0 + j / 64.0)
                  + np.float32(2.0 ** -8))
            for j in range(1, 1 + N_ACT)]
    return np.tile(np.array(cols, np.float32), (128, 1))


def _wones_np() -> np.ndarray:
    w = np.zeros((128, 32 * EDGE_BLOCK), ml_dtypes.bfloat16)
    for e in range(EDGE_BLOCK):
        for i in range(4):
            w[32 * i:32 * (i + 1), 32 * e + 4 * e + i] = 1
    return w


def _make_in_maps(input: np.ndarray, target: np.ndarray):
    inp = np.asarray(input, np.float32)
    tgt = np.asarray(target, np.float32)
    bias = _bias_np()
    wones = _wones_np()
    in_maps = []
    for c in range(N_CORES):
        sl = slice(c * B_LOC, (c + 1) * B_LOC)
        in_maps.append({"x": _pack_core(inp[sl], tgt[sl]), "bias_a": bias,
                        "wones": wones})
    return in_maps


def kernel(input: np.ndarray, target: np.ndarray) -> np.ndarray:
    from concourse.bass_utils import run_bass_kernel_spmd

    nc = _get_nc()
    res = run_bass_kernel_spmd(
        nc, _make_in_maps(input, target), core_ids=list(range(N_CORES)))
    return np.asarray(_counts_to_loss(res.results), np.float32)


# revision 28
# speedup vs baseline: 1.3452x; 1.3452x over previous
"""ColorHistogramLoss Trainium2 kernel.

Problem: loss = mean(|hist(input) - hist(target)|) with 64-bin histograms
per (batch, channel) over [-1, 1), inputs [32, 3, 512, 512] f32.

Strategy (8 cores, data-parallel over batch, 4 batches/core):
  - Binning: w = bf16_rne(v*(63/128) + (191/128 - 2^-8)). The -2^-8 pre-bias
    turns bf16 round-to-nearest into floor onto the 2^-7 grid of [1,2), so
    (w >= 1 + j/64) reproduces searchsorted binning exactly (boundary-rounding
    differences ~1e-5 of elements, loss rel-err ~1e-4).
  - CDF counts per edge j, split across three engines (HW-measured balance):
      ACT: activation(Sign, bias=-(1+j/64-2^-9), accum_out) — fused count.
      DVE: tensor_scalar(is_ge, imm) WITHOUT accum (keeps the 4x perf mode;
           the fused accum variant drops to 2x) producing a 0/1 bf16 mask.
      PE:  reduces each mask: matmul(ones-indicator [128,4] x mask chunk
           [128,512]) accumulated over 16 chunks into PSUM [4 imgs, 512];
           8 edges share one [32,512] PSUM tile (disjoint 4-row slices).
      A tiny DVE accum op then reduces each PSUM tile to per-(edge,img)
      counts. Exact integer arithmetic throughout (f32 PSUM, counts < 2^24).
  - Host differentiates the CDF and does the tiny final reduction.
  - Layout: 24 channel-images per core (4 batches x 3 ch x 2 tensors),
    packed 4 per SBUF tile as [128, 8192] f32 -> 6 group tiles.
"""

import numpy as np
import ml_dtypes

BINS = 64
N_CORES = 8
B, C, H, W = 32, 3, 512, 512
NPIX = H * W                  # 262144 per channel-image
B_LOC = B // N_CORES          # 4
IMGS = 2 * B_LOC * C          # 24 channel-images per core
PACK = 4                      # channel-images per SBUF group tile
GROUPS = IMGS // PACK         # 6
PART_PER_IMG = 128 // PACK    # 32 partitions per image
FD = NPIX // PART_PER_IMG     # 8192 free-dim elements per partition

SCALE = float(np.float32(63.0 / 128.0))              # exact in f32
BIAS2 = float(np.float32(191.0 / 128.0) - np.float32(2.0 ** -8))

# edges j=1..63; ACT (Sign+accum) takes the first N_ACT, the rest go
# DVE-mask + PE-reduce. HW A/B (For_i slope timing; axon dispatch jitter
# makes single-shot timing useless): fused-accum DVE runs ~8.3us/tile (2x
# cap + DRAIN) vs ~4.2us for mask-only (4x), ACT ~10us/tile, PE reduction
# ~3.4us/mask — so PE absorbs the accumulation and DVE produces masks.
# Full-workload slopes: all-fused 16/47 split 2.87ms, 28/35 fused 1.86ms,
# PE-reduce n_act=8 1.21ms; + ACT-on-raw-input + conv/readouts on ACT +
# masks in 2 half-tiles (better DVE->PE overlap) 1.05ms.
N_ACT = 8
MASK_SPLIT = 2                # mask half-tiles per edge (DVE->PE overlap)
EDGE_BLOCK = 8                # PE-routed edges per PSUM tile (4 rows each)
N_PE = BINS - 1 - N_ACT       # 44
N_BLOCKS = (N_PE + EDGE_BLOCK - 1) // EDGE_BLOCK     # 6
CHUNK = 512                   # matmul moving free size
N_CHUNKS = FD // CHUNK        # 16

_cache = {}


def _build():
    from concourse import bacc
    import concourse.mybir as mybir
    from concourse.tile import TileContext

    f32 = mybir.dt.float32
    bf16 = mybir.dt.bfloat16

    nc = bacc.Bacc("TRN2", target_bir_lowering=False, debug=False,
                   num_devices=N_CORES)
    x = nc.declare_dram_parameter("x", [GROUPS, 128, FD], f32, isOutput=False)
    bias_a = nc.declare_dram_parameter(
        "bias_a", [128, N_ACT], f32, isOutput=False)
    # 8 stationary variants [128, 32]: variant e holds the 4 per-image
    # indicator columns at columns 4e..4e+4 (zeros elsewhere), so every
    # matmul targets the full [32, CHUNK] PSUM tile (PE requires output
    # base partition 0/32/64) and cross-edge rows just accumulate zeros.
    wones_p = nc.declare_dram_parameter(
        "wones", [128, 32 * EDGE_BLOCK], mybir.dt.bfloat16, isOutput=False)
    # counts_a[g, p, j] = sum(sign(w - (1 + j/64 - 2^-9))) for ACT-owned j
    counts_a = nc.declare_dram_parameter(
        "counts_a", [GROUPS, 128, BINS], f32, isOutput=True)
    # counts_pe[g, 4*e+i, b] = #{w >= edge(block b, slot e)} in image i
    counts_pe = nc.declare_dram_parameter(
        "counts_pe", [GROUPS, 32, N_BLOCKS], f32, isOutput=True)

    edges = list(range(1, BINS))
    edges_act = edges[:N_ACT]
    edges_pe = edges[N_ACT:]

    with TileContext(nc) as tc, tc.tile_pool(name="p", bufs=2) as pool, \
            tc.psum_pool(name="pp", bufs=2) as ppool:
        bt = pool.tile([128, N_ACT], f32, tag="bt")
        nc.sync.dma_start(out=bt[:], in_=bias_a[:])
        wones = pool.tile([128, 32 * EDGE_BLOCK], bf16, tag="wones")
        nc.sync.dma_start(out=wones[:], in_=wones_p[:])
        HFD = FD // MASK_SPLIT
        HCH = HFD // CHUNK
        for g in range(GROUPS):
            vt = pool.tile([128, FD], f32, tag="vt")
            nc.sync.dma_start(out=vt[:], in_=x[g])
            # conv on ACT (DVE is the bottleneck engine): w = bf16(v*S + B)
            w = pool.tile([128, FD], bf16, tag="w")
            nc.scalar.activation(
                out=w[:], in_=vt[:],
                func=mybir.ActivationFunctionType.Copy,
                bias=BIAS2, scale=SCALE)

            cnt_a = pool.tile([128, BINS], f32, tag="cnta")
            nc.gpsimd.memset(cnt_a[:], 0.0)
            mask_a = pool.tile([128, FD], bf16, tag="maska")

            # ACT compares raw v directly: sign(v*S + (B - t_j + 2^-8)) == 1
            # iff RNE(v*S + B) >= t_j (tie at t_j - 2^-8 rounds up to the
            # even mantissa t_j), matching the w-grid compare exactly.
            # Fused ACT edges are emitted INTERLEAVED between PE blocks (2
            # per block) so each block's PSUM readout lands on the in-order
            # ACT queue before the PE needs that buffer back — emitting all
            # fused edges up front would park ~80us of ACT work ahead of
            # readout 0 and stall the PE/DVE pipeline every group.
            fused = list(enumerate(edges_act))

            def emit_fused(n):
                for k, j in fused[:n]:
                    nc.scalar.activation(
                        out=mask_a[:], in_=vt[:],
                        func=mybir.ActivationFunctionType.Sign,
                        bias=bt[:, k:k + 1], scale=SCALE,
                        accum_out=cnt_a[:, j:j + 1])
                del fused[:n]

            cnt_pe = pool.tile([32, N_BLOCKS], f32, tag="cntpe")
            scratch = pool.tile([32, CHUNK], bf16, tag="scratch")
            for b in range(N_BLOCKS):
                blk = edges_pe[b * EDGE_BLOCK:(b + 1) * EDGE_BLOCK]
                ps = ppool.tile([32, CHUNK], f32, tag="ps", name="ps")
                for e, j in enumerate(blk):
                    # One accumulation group per edge-half (the scheduler
                    # treats a group as atomic on PE; a group spanning all 8
                    # edges would wait on masks produced mid-group ->
                    # deadlock). Groups after the first accumulate onto ps
                    # via start=False; cross-edge rows only ever add zeros.
                    # Masks are produced in MASK_SPLIT half-tiles so the PE
                    # starts reducing half h while DVE writes half h+1.
                    for h in range(MASK_SPLIT):
                        mask = pool.tile([128, HFD], bf16, tag=f"mask{h}",
                                         name=f"mask{h}")
                        nc.vector.tensor_scalar(
                            out=mask[:], in0=w[:, h * HFD:(h + 1) * HFD],
                            scalar1=float(1.0 + j / 64.0), scalar2=None,
                            op0=mybir.AluOpType.is_ge,
                            op1=mybir.AluOpType.bypass)
                        for c in range(HCH):
                            nc.tensor.matmul(
                                out=ps[:, :],
                                lhsT=wones[:, 32 * e:32 * (e + 1)],
                                rhs=mask[:, c * CHUNK:(c + 1) * CHUNK],
                                start=(e == 0 and h == 0 and c == 0),
                                stop=(h == MASK_SPLIT - 1 and c == HCH - 1),
                                skip_group_check=True)
                emit_fused(2)
                # rows 4e..4e+4 of ps hold per-image chunk-partial sums for
                # edge blk[e]; reduce the 512 columns into one count each
                # (on ACT — keeps the bottleneck DVE mask-only).
                nrows = 4 * len(blk)
                nc.scalar.activation(
                    out=scratch[:nrows, :], in_=ps[:nrows, :],
                    func=mybir.ActivationFunctionType.Copy,
                    bias=0.0, scale=1.0,
                    accum_out=cnt_pe[:nrows, b:b + 1])
            emit_fused(len(fused))
            nc.sync.dma_start(out=counts_a[g], in_=cnt_a[:])
            nc.sync.dma_start(out=counts_pe[g], in_=cnt_pe[:])
    nc.finalize()
    return nc


def _get_nc():
    if "nc" not in _cache:
        _cache["nc"] = _build()
    return _cache["nc"]


def _pack_core(inp_c: np.ndarray, tgt_c: np.ndarray) -> np.ndarray:
    """[4,3,512,512] x2 f32 -> [GROUPS, 128, FD]; image i = t*12 + b*3 + c."""
    imgs = np.concatenate(
        [inp_c.reshape(B_LOC * C, NPIX), tgt_c.reshape(B_LOC * C, NPIX)], axis=0)
    return np.ascontiguousarray(
        imgs.reshape(GROUPS, PACK, PART_PER_IMG, FD).reshape(GROUPS, 128, FD))


def _counts_to_loss(results) -> np.float32:
    """results: list of 8 dicts with counts_a [G,128,BINS], counts_pe
    [G,32,N_BLOCKS]."""
    total = np.float64(0.0)
    for c in range(N_CORES):
        ca = np.asarray(results[c]["counts_a"], np.float64)
        cpe = np.asarray(results[c]["counts_pe"], np.float64)
        ca = ca.reshape(GROUPS, PACK, PART_PER_IMG, BINS).sum(axis=2)
        ca = ca.reshape(IMGS, BINS)
        cdf = np.zeros((IMGS, BINS), np.float64)
        cdf[:, 0] = NPIX
        for j in range(1, 1 + N_ACT):
            cdf[:, j] = (NPIX + ca[:, j]) / 2.0   # sign-sum -> count_ge
        for k in range(N_PE):
            j = 1 + N_ACT + k
            b, e = divmod(k, EDGE_BLOCK)
            for g in range(GROUPS):
                for i in range(PACK):
                    cdf[g * PACK + i, j] = cpe[g, 4 * e + i, b]
        counts = np.empty((IMGS, BINS), np.float64)
        counts[:, :-1] = cdf[:, :-1] - cdf[:, 1:]
        counts[:, -1] = cdf[:, -1]
        hist = counts / NPIX   # [24, 64]; images 0..11 = input, 12..23 = target
        h_in = hist[: B_LOC * C].reshape(B_LOC, C * BINS)
        h_tg = hist[B_LOC * C:].reshape(B_LOC, C * BINS)
        total += np.abs(h_in - h_tg).sum()
    return np.float32(total / (B * C * BINS))


def _bias_np() -> np.ndarray:
    # ACT reads raw v: sign(v*SCALE + bias_j) >= 0  <=>  x >= t_j - 2^-8
    # with x = v*SCALE + BIAS2, i.e. RNE_bf16(x) >= t_j (tie rounds up).
    cols = [float(np.float32(BIAS2) - np.float32(1.0 + j / 64.0)
                  + np.float32(2.0 ** -8))
            for j in range(1, 1 + N_ACT)]
    return np.tile(np.array(cols, np.float32), (128, 1))


def _wones_np() -> np.ndarray:
    w = np.zeros((128, 32 * EDGE_BLOCK), ml_dtypes.bfloat16)
    for e in range(EDGE_BLOCK):
        for i in range(4):
            w[32 * i:32 * (i + 1), 32 * e + 4 * e + i] = 1
    return w


def _make_in_maps(input: np.ndarray, target: np.ndarray):
    inp = np.asarray(input, np.float32)
    tgt = np.asarray(target, np.float32)
    bias = _bias_np()
    wones = _wones_np()
    in_maps = []
    for c in range(N_CORES):
        sl = slice(c * B_LOC, (c + 1) * B_LOC)
        in_maps.append({"x": _pack_core(inp[sl], tgt[sl]), "bias_a": bias,
                        "wones": wones})
    return in_maps


def kernel(input: np.ndarray, target: np.ndarray) -> np.ndarray:
    from concourse.bass_utils import run_bass_kernel_spmd

    nc = _get_nc()
    res = run_bass_kernel_spmd(
        nc, _make_in_maps(input, target), core_ids=list(range(N_CORES)))
    return np.asarray(_counts_to_loss(res.results), np.float32)
